# revision 98
# baseline (speedup 1.0000x reference)
"""Trainium2 Bass kernel for the circular drift-diffusion loss (batched expm).

Reference computes  loss = -mean_b log(relu(e_{idx_b}^T expm(t_b*A) p0_b) + eps)
with A a fixed 360x360 circular advection-diffusion operator, t_b in [0,1000),
p0_b a von Mises density, over a batch of 256.

Algorithm (per core; batch sharded 32/core over 8 cores):
  * Quantize t_b = m_b*T0 + r_b with T0 = 1000/2^K, m_b < 2^K.
  * Build propagator chain M_j = expm(2^j*T0*A) once by repeated squaring
    (prelude: ascending Taylor at T0/2^PRE_SQ, then PRE_SQ squarings -> M_0;
    then K-1 squarings).  A squaring is 9 f32 matmuls for S = M@M plus 9 PE
    transposes for S^T (needed as the next stationary operand).  K and the
    Taylor degrees are chosen at runtime from ||A||_inf so both
    heavy-diffusion and near-advection inputs are optimal.
  * Apply bits of m_b as masked batched matvecs: Q <- bit_j ? M_j Q : Q.
  * Residual: Q <- Taylor_DEG_R(r_b A) Q (Horner, per-sample scalar folded
    into host-precomputed r/k coefficient tables).
  * p0 built on device (folded poly cos + Exp activation), selection via
    one-hot + PE column-sum, loss terms via Ln activation.
Everything O(n^2)+ runs on device; host does only index/bit/layout glue and
the tridiagonal operator assembly (exactly replicating the reference's f32
evo_mat construction).
"""

import math

import numpy as np

# ---------------- static problem constants (hardcoded per contract) ----------
N = 360            # color mesh size
P = 120            # partition chunk (N = 3*P)
NCH = 3            # chunks
B = 256            # total batch
NCORES = 8
BL = B // NCORES   # per-core batch
T_MAX = 1000.0
KAPPA = 400.0      # 1/SIGMA_INIT^2
EPS = 1e-5
TWO_PI = 6.283185307179586
# ln(1/(2*pi*i0e(400)))  [i0e(400) = 0.019953356281939987]
LNC = 2.076480848703078
# cos(sqrt(u)) on u in [0, pi^2] (|delta| folded to [0,pi]), power basis c0..c8
COS_COEF = [1.00000000e+00, -5.00000000e-01, 4.16666666e-02, -1.38888885e-03,
            2.48015646e-05, -2.75566515e-07, 2.08651966e-09, -1.13535474e-11,
            4.13131734e-14]

_COMPILED = {}


def _taylor_deg(x, tol, lo):
    """Smallest d with x^(d+1)/(d+1)! < tol."""
    d = lo
    term = x ** (d + 1) / math.factorial(d + 1)
    while term > tol and d < 40:
        d += 1
        term *= x / (d + 1)
    return d


def _plan(anorm):
    """Choose (k_bits, deg_p, deg_r) from ||A||_inf.  The time grid is
    T0 = T_MAX/2^k_bits, chosen so the prelude Taylor converges fast at T0;
    every squaring level applies one bit of the quantized delay."""
    xa = T_MAX * float(anorm)
    if xa <= 0.0:
        return 2, 4, 3
    k0 = max(2, min(16, math.ceil(math.log2(max(xa / 0.9, 2.0)))))

    def degrees(k):
        x0 = xa / (1 << k)
        # prelude truncation amplifies roughly 2^(k/2) through the
        # squarings, so its tolerance adapts to chain depth; the residual
        # Taylor is applied once (no amplification) and tolerates more.
        # Tolerances sized so truncation stays well under the fp32r
        # rounding noise of the chain (~3e-3 at the loss level).
        tol_p = min(max(3e-3 / 2 ** (k / 2), 5e-8), 2e-4)
        return _taylor_deg(x0, tol_p, 3), _taylor_deg(x0, 3e-4, 2)

    # pick k by explicit cost minimization with measured per-stage costs
    # (fp32r: chain level ~2.7us, prelude step ~2.3us, taylor step ~0.3us)
    best = None
    for k in range(max(2, k0 - 2), min(16, k0 + 3) + 1):
        dp, dr = degrees(k)
        cost = (k - 1) * 2.7 + (dp - 1) * 2.3 + dr * 0.3
        if best is None or cost < best[0]:
            best = (cost, k, dp, dr)
    _, k, deg_p, deg_r = best
    # the kernel merges residual steps j=deg_r..2 into prelude steps
    # k=3..deg_p, so it needs at least deg_r-1 merged slots
    return k, max(deg_p, deg_r + 1), deg_r


def _build_bass(k_bits, deg_p, deg_r):
    """Construct the Bass program (SPMD; identical on all 8 cores)."""
    import concourse.tile as tile
    from concourse import bacc, mybir

    F32 = mybir.dt.float32
    F32R = mybir.dt.float32r
    AF = mybir.ActivationFunctionType
    OP = mybir.AluOpType

    nc = bacc.Bacc("TRN2", target_bir_lowering=False, debug=False)

    def din(name, shape, dt=F32):
        return nc.dram_tensor(name, shape, dt, kind="ExternalInput").ap()

    d_x = din("x", [N, N], F32R)   # X = T0*A
    d_xi = din("xi", [N, N], F32R)  # I + X (prelude Taylor start)
    # aux = [q0 | masks | rdk] in one DMA: every HWDGE DMA holds the
    # shared DGE ~625ns, so inputs are consolidated
    AUXW = NCH * BL + k_bits * BL + deg_r * BL
    d_aux = din("aux", [P, AUXW], F32R)
    # four top-bit candidate density blocks out; per-sample selection by
    # the top two delay bits + log + mean happen on host
    d_out = nc.dram_tensor("v", [P, 4 * NCH * BL], F32R,
                           kind="ExternalOutput").ap()

    with tile.TileContext(nc) as tc:
        with (
            tc.tile_pool(name="const", bufs=1) as cpool,
            tc.tile_pool(name="mats", bufs=3) as mpool,
            tc.tile_pool(name="qp", bufs=2) as qpool,
            tc.tile_pool(name="vp", bufs=3) as vpool,
            tc.tile_pool(name="tp", bufs=4) as tpool,
            tc.tile_pool(name="psb", bufs=5, space="PSUM") as psb,
            tc.tile_pool(name="pss", bufs=3, space="PSUM") as pss,
        ):
            # ---- constants ------------------------------------------------
            # chunk 0 of x lands as [P,P] pieces on parallel queues so the
            # on-device XT transposes (and hence the first prelude matmul)
            # start early; other chunks whole
            # chunk 0 first (it gates the XT transposes), chunks 1-2 merged
            XN = cpool.tile([P, NCH * N], F32R, tag="x")
            XT = cpool.tile([P, NCH * N], F32R, tag="xt")
            # HWDGE serializes DMAs across queues (~625ns each), so inputs
            # go on one queue in deadline order: x0, x12, aux, xi
            nc.sync.dma_start(XN[:, 0:N], d_x[0:P, :])
            nc.sync.dma_start(
                XN[:, N:3 * N].rearrange("p (c n) -> p c n", c=2),
                d_x[P:3 * P, :].rearrange("(c p) n -> p c n", c=2))
            AUX = cpool.tile([P, AUXW], F32R, tag="aux")
            nc.sync.dma_start(AUX[:], d_aux[:])
            Q = AUX[:, 0:NCH * BL]
            MSK = AUX[:, NCH * BL: (NCH + k_bits) * BL]
            RDK = AUX[:, (NCH + k_bits) * BL: AUXW]
            # identity for PE transposes (fp32r producers: memset can't
            # write fp32r, so build in f32 then copy)
            E120S = cpool.tile([P, P], F32, tag="e120s")
            nc.vector.memset(E120S[:], 1.0)
            nc.gpsimd.affine_select(
                E120S[:], E120S[:], pattern=[[1, P]], compare_op=OP.is_equal,
                fill=0.0, base=0, channel_multiplier=-1,
            )
            E120 = cpool.tile([P, P], F32R, tag="e120")
            nc.vector.tensor_copy(E120[:], E120S[:])
            # X^T built on device: block (c,i) of XT = transpose of X block
            # (i,c); group i reads the contiguous chunk i of XN
            for i in range(NCH):
                pst = psb.tile([P, N], F32, tag="sq")
                for c in range(NCH):
                    nc.tensor.transpose(
                        pst[:, c * P:(c + 1) * P].bitcast(F32R),
                        XN[:, i * N + c * P: i * N + c * P + P],
                        E120[:],
                    )
                if i % 2 == 0:
                    nc.vector.tensor_copy(XT[:, i * N:(i + 1) * N], pst[:])
                else:
                    nc.scalar.copy(XT[:, i * N:(i + 1) * N], pst[:])

            W = N + BL  # merged chunk width: [M_c | Q_c]

            def mm_group(ps, lhsT_tile, rhs_tile, i, rhs_w, rhs_stride=None):
                # lhsT tiles are i-major: block (c, i) at col i*N + c*P, so
                # output chunk i depends on one contiguous lhsT region
                rs = rhs_w if rhs_stride is None else rhs_stride
                for c in range(NCH):
                    nc.tensor.matmul(
                        ps[:],
                        lhsT=lhsT_tile[:, i * N + c * P: i * N + c * P + P],
                        rhs=rhs_tile[:, c * rs: c * rs + rhs_w],
                        start=(c == 0), stop=(c == NCH - 1),
                    )

            def copy_out(dst_ap, ps, idx, small=False):
                # PSUM->SBUF copies: big 1/4 DVE 3/4 ACT; small (transpose
                # blocks) 1/2-1/2 -- ACT's fixed overhead dominates there
                mod = 2 if small else 4
                if idx % mod == 0:
                    nc.vector.tensor_copy(dst_ap, ps[:])
                else:
                    nc.scalar.copy(dst_ap, ps[:])

            # p0 lives in AUX (Q slice); aux/xi DMAs are emitted after the
            # first XT copies so they don't delay them in the queues

            # ---- prelude: ascending Taylor S = I + sum X^k/k!, with the
            # residual Taylor on p0 (V = Q + rdk_j*(X V), j=deg_r..1, which
            # commutes with the bit applies) MERGED into the prelude
            # matmuls as 32 extra moving columns.  T tiles are MQ-shaped
            # ([T_c | V_c]) so each step's psum carries both products and
            # the V drains never wait on a separate matmul set.  I+X comes
            # from the host, saving the identity memset and 3 wide adds.
            S = mpool.tile([P, NCH * W], F32R, tag="M")
            nc.sync.dma_start(
                S[:].rearrange("p (c w) -> p c w", c=NCH)[:, :, 0:N],
                d_xi[:, :].rearrange("(c p) n -> p c n", c=NCH))
            # residual Taylor state: V = Q + rdk_j*(X V), j=deg_r..1,
            # evaluated as standalone narrow matmul sets deferred by one
            # prelude step so the DVE drains never head-of-line block
            tj = {"j": deg_r, "V": Q, "stride": BL, "off": 0}

            def taylor_step(to_dst=False):
                j = tj["j"]
                if j < 1:
                    return
                Vn = None if to_dst else vpool.tile([P, NCH * BL], F32R,
                                                    tag="V")
                rd = RDK[:, (j - 1) * BL: j * BL]
                for i in range(NCH):
                    ps = pss.tile([P, BL], F32, tag="ap")
                    for c in range(NCH):
                        nc.tensor.matmul(
                            ps[:],
                            lhsT=XT[:, i * N + c * P: i * N + c * P + P],
                            rhs=tj["V"][:, c * tj["stride"] + tj["off"]:
                                        c * tj["stride"] + tj["off"] + BL],
                            start=(c == 0), stop=(c == NCH - 1),
                        )
                    # final step lands straight in the MQ tile's Q slots
                    vs = (S[:, i * W + N: (i + 1) * W] if to_dst
                          else Vn[:, i * BL:(i + 1) * BL])
                    nc.vector.tensor_tensor(vs, ps[:], rd, op=OP.mult)
                    nc.gpsimd.tensor_tensor(vs, vs,
                                            Q[:, i * BL:(i + 1) * BL],
                                            op=OP.add)
                if to_dst:
                    tj["j"] = 0
                else:
                    tj.update(j=j - 1, V=Vn[:], stride=BL, off=0)

            T = XN
            for k in range(2, deg_p + 1):
                # taylor matmuls FIRST on PE: their psum is ready before
                # this step's, so the taylor drain on DVE never delays the
                # next gating drain0
                if k > 2:
                    taylor_step(to_dst=(tj["j"] == 1))
                last = k == deg_p
                Tn = mpool.tile([P, NCH * N], F32R, tag="T")
                for i in range(NCH):
                    ps = psb.tile([P, N], F32, tag="sq")
                    mm_group(ps, XT, T, i, N)
                    # drain i=0 on DVE (it gates the next step's first
                    # matmul), the rest on ACT
                    if i == 0:
                        nc.vector.tensor_scalar(Tn[:, i * N:(i + 1) * N],
                                                ps[:], 1.0 / k, None,
                                                op0=OP.mult)
                    else:
                        nc.scalar.mul(Tn[:, i * N:(i + 1) * N], ps[:], 1.0 / k)
                    # S accumulation off the critical path on Pool, except
                    # the last step whose adds gate the first squaring
                    seng = nc.vector if (last and i != 1) else nc.gpsimd
                    seng.tensor_tensor(S[:, i * W: i * W + N],
                                       S[:, i * W: i * W + N],
                                       Tn[:, i * N:(i + 1) * N],
                                       op=OP.add)
                T = Tn

            while tj["j"] >= 1:
                taylor_step(to_dst=(tj["j"] == 1))

            ST = mpool.tile([P, NCH * N], F32R, tag="MT")

            tr_rot = {"k": 0}

            def transpose_group(MTt, Mt, i):
                # i-major MT: for output chunk i the 3 transposes read the
                # contiguous blocks (i, c) of M and land in ONE [P, N] psum
                # tile drained by a single wide copy; the next squaring's
                # chunk-0 matmuls depend only on the first copy
                pst = psb.tile([P, N], F32, tag="sq")
                for c in range(NCH):
                    nc.tensor.transpose(
                        pst[:, c * P:(c + 1) * P].bitcast(F32R),
                        Mt[:, i * W + c * P: i * W + c * P + P],
                        E120[:],
                    )
                dst = MTt[:, i * N:(i + 1) * N]
                k = tr_rot["k"] = tr_rot["k"] + 1
                # PSUM drains: only DVE/ACT can read PSUM
                if k % 2 == 0:
                    nc.vector.tensor_copy(dst, pst[:])
                else:
                    nc.scalar.copy(dst, pst[:])

            def transpose_mq(MTt, Mt):
                for i in range(NCH):
                    transpose_group(MTt, Mt, i)

            transpose_mq(ST, S)
            M, MT = S, ST

            def square(Mc, MTc, bit=None):
                # Sn = Mc@Mc; if bit is not None also compute Mc@Q (merged
                # columns) and blend it into Sn's Q slot under the bit mask
                # (arithmetic blend: copy_predicated can't produce fp32r).
                Sn = mpool.tile([P, NCH * W], F32R, tag="M")
                STn = mpool.tile([P, NCH * N], F32R, tag="MT")
                wid = N if bit is None else W
                for i in range(NCH):
                    ps = psb.tile([P, wid], F32, tag="sq")
                    mm_group(ps, MTc, Mc, i, wid, rhs_stride=W)
                    copy_out(Sn[:, i * W: i * W + N], ps[:, :N], i)
                    if bit is not None:
                        qold = Mc[:, i * W + N: (i + 1) * W]
                        dq = tpool.tile([P, BL], F32, tag="blend")
                        nc.vector.tensor_tensor(dq[:], ps[:, N:W], qold,
                                                op=OP.subtract)
                        nc.vector.tensor_tensor(
                            dq[:], dq[:], MSK[:, bit * BL:(bit + 1) * BL],
                            op=OP.mult)
                        nc.gpsimd.tensor_tensor(
                            Sn[:, i * W + N: (i + 1) * W], dq[:], qold,
                            op=OP.add)
                transpose_mq(STn, Sn)
                return Sn, STn

            # ---- merged bit applies + chain squarings ---------------------
            # level j squares M (= expm(2^j T0 A)) and applies bit j of the
            # quantized delay to Q in the same matmul set.  The top TWO bits
            # need no further squaring: bit k-2 is a single apply of M_{k-2}
            # and bit k-1 a double apply (M_{k-1} Q = M_{k-2} (M_{k-2} Q)),
            # which is ~2x cheaper than materializing M_{k-1}.
            for j in range(k_bits - 2):
                M, MT = square(M, MT, bit=j)

            QW = NCH * BL

            def apply_wide(q_rhs, rhs_stride, rhs_off):
                # one [P, NCH*BL] psum: chunk i's accumulation lands in
                # slice i, so the blend afterwards is one set of wide ops
                ps = pss.tile([P, QW], F32, tag="ap")
                for i in range(NCH):
                    for c in range(NCH):
                        nc.tensor.matmul(
                            ps[:, i * BL:(i + 1) * BL],
                            lhsT=MT[:, i * N + c * P: i * N + c * P + P],
                            rhs=q_rhs[:, c * rhs_stride + rhs_off:
                                      c * rhs_stride + rhs_off + BL],
                            start=(c == 0), stop=(c == NCH - 1),
                        )
                return ps

            def ccb(ap):
                return ap.rearrange("p (c b) -> p c b", c=NCH)

            # top-2-bit candidates: YOUT = [Q' | MQ' | M^2Q' | M^3Q'] with
            # Q' the fully bit-merged density; the host selects per sample
            # by the top two delay bits, so no device-side blending
            qmq = M[:].rearrange("p (c w) -> p c w", c=NCH)[:, :, N:W]
            YOUT = qpool.tile([P, 4 * QW], F32R, tag="yout")
            nc.gpsimd.tensor_copy(ccb(YOUT[:, 0:QW]), qmq)
            for a in range(1, 4):
                if a == 1:
                    ps = apply_wide(M, W, N)
                else:
                    ps = apply_wide(YOUT, BL, (a - 1) * QW)
                if a == 2:
                    nc.scalar.copy(YOUT[:, a * QW:(a + 1) * QW], ps[:])
                else:
                    nc.vector.tensor_copy(YOUT[:, a * QW:(a + 1) * QW], ps[:])
                if a == 2:
                    # ship the first three candidates while Y3 computes
                    nc.sync.dma_start(d_out[:, 0:3 * QW], YOUT[:, 0:3 * QW])
            nc.sync.dma_start(d_out[:, 3 * QW:4 * QW],
                              YOUT[:, 3 * QW:4 * QW])

    nc.compile()
    return nc


def _host_prep(c_mesh, gtheta, sigma_diff, init_color, delay_t, report_color):
    """Host-side glue: operator assembly (replicating reference f32 ops),
    plan selection, and per-core index/bit/layout arrays."""
    f32 = np.float32
    c = np.asarray(c_mesh, dtype=f32)
    g = np.asarray(gtheta, dtype=f32)
    s = np.asarray(sigma_diff, dtype=f32)[0]
    init = np.asarray(init_color, dtype=f32)
    t = np.asarray(delay_t, dtype=f32)
    rep = np.asarray(report_color, dtype=f32)

    d = (c[1] - c[0]).astype(f32)
    eye = np.eye(N, dtype=f32)
    up = np.roll(eye, -1, axis=1)
    dn = np.roll(eye, 1, axis=1)
    D1 = ((up - dn) / (f32(2.0) * d)).astype(f32)
    D2 = ((up - f32(2.0) * eye + dn) / (d * d)).astype(f32)
    A = ((s ** f32(2.0)) / f32(2.0) * D2 - D1 * g[None, :]).astype(f32)

    anorm = np.abs(A.astype(np.float64)).sum(axis=1).max()
    k_bits, deg_p, deg_r = plan = _plan(anorm)
    T0 = T_MAX / (1 << k_bits)
    X = (A * f32(T0)).astype(f32)

    m = np.floor(t.astype(np.float64) / T0).astype(np.int64)
    m = np.clip(m, 0, (1 << k_bits) - 1)
    r = (t.astype(np.float64) - m * T0) / T0  # in X = T0*A units
    bits = ((m[:, None] >> np.arange(k_bits)[None, :]) & 1)     # [B, K]
    idx = np.argmin(np.abs(c[None, :] - rep[:, None]), axis=1)

    # p0 host-side (O(B*n) glue, like the one-hot/argmin prep): von Mises
    # density replicating the reference's f32 formula
    z = np.cos(c[None, :].astype(np.float64)
               - init[:, None].astype(np.float64)) - 1.0
    p0 = (np.exp(KAPPA * z + LNC)).astype(f32)          # [B, n]

    shared = {
        "x": X,
        "xi": (np.eye(N, dtype=f32) + X).astype(f32),
    }
    in_maps = []
    for core in range(NCORES):
        sl = slice(core * BL, (core + 1) * BL)
        # Q layout [P, NCH*BL]: chunk c at cols [c*BL:(c+1)*BL], Q[p,c*BL+b]
        # = p0[b, c*P+p]
        q0 = np.ascontiguousarray(
            p0[sl].reshape(BL, NCH, P).transpose(2, 1, 0).reshape(P, NCH * BL))
        msk = bits[sl].T.reshape(1, k_bits * BL).astype(f32)
        rdk = np.empty((deg_r, BL), f32)
        for k in range(1, deg_r + 1):
            rdk[k - 1] = (r[sl] / k).astype(f32)
        rdk = rdk.reshape(1, deg_r * BL)
        aux = np.concatenate([
            q0,
            np.broadcast_to(msk, (P, k_bits * BL)),
            np.broadcast_to(rdk, (P, deg_r * BL)),
        ], axis=1).astype(f32)
        in_maps.append(dict(shared, aux=np.ascontiguousarray(aux)))
    # per-sample candidate index = #applies of M_{k-2} from the top 2 bits
    topsel = bits[:, k_bits - 2] + 2 * bits[:, k_bits - 1]
    return plan, in_maps, (idx, topsel)


def _get_nc(plan):
    if plan not in _COMPILED:
        _COMPILED[plan] = _build_bass(*plan)
    return _COMPILED[plan]


def kernel(**inputs):
    from concourse.bass_utils import run_bass_kernel_spmd

    plan, in_maps, (idx, topsel) = _host_prep(
        inputs["c_mesh"], inputs["gtheta"], inputs["sigma_diff"],
        inputs["init_color"], inputs["delay_t"], inputs["report_color"],
    )
    nc = _get_nc(plan)
    res = run_bass_kernel_spmd(nc, in_maps, list(range(NCORES)))
    # host-side candidate pick + selection + log + mean (O(B) glue):
    # v[p, a*NCH*BL + c*BL + b] holds candidate a of p1[b, c*P+p]
    QW = NCH * BL
    terms = np.empty(B, np.float64)
    for core in range(NCORES):
        v = np.asarray(res.results[core]["v"])
        for b in range(BL):
            g = core * BL + b
            ix = idx[g]
            p_sel = np.float32(
                v[ix % P, topsel[g] * QW + (ix // P) * BL + b])
            terms[g] = np.log(np.float32(max(p_sel, 0.0) + EPS))
    loss = -np.mean(terms)
    return np.asarray(loss, dtype=np.float32)



# revision 101
# speedup vs baseline: 1.0509x; 1.0509x over previous
"""Trainium2 Bass kernel for the circular drift-diffusion loss (batched expm).

Reference computes  loss = -mean_b log(relu(e_{idx_b}^T expm(t_b*A) p0_b) + eps)
with A a fixed 360x360 circular advection-diffusion operator, t_b in [0,1000),
p0_b a von Mises density, over a batch of 256.

Algorithm (per core; batch sharded 32/core over 8 cores):
  * Quantize t_b = m_b*T0 + r_b with T0 = 1000/2^K, m_b < 2^K.
  * Build propagator chain M_j = expm(2^j*T0*A) once by repeated squaring
    (prelude: ascending Taylor at T0/2^PRE_SQ, then PRE_SQ squarings -> M_0;
    then K-1 squarings).  A squaring is 9 f32 matmuls for S = M@M plus 9 PE
    transposes for S^T (needed as the next stationary operand).  K and the
    Taylor degrees are chosen at runtime from ||A||_inf so both
    heavy-diffusion and near-advection inputs are optimal.
  * Apply bits of m_b as masked batched matvecs: Q <- bit_j ? M_j Q : Q.
  * Residual: Q <- Taylor_DEG_R(r_b A) Q (Horner, per-sample scalar folded
    into host-precomputed r/k coefficient tables).
  * p0 built on device (folded poly cos + Exp activation), selection via
    one-hot + PE column-sum, loss terms via Ln activation.
Everything O(n^2)+ runs on device; host does only index/bit/layout glue and
the tridiagonal operator assembly (exactly replicating the reference's f32
evo_mat construction).
"""

import math

import numpy as np

# ---------------- static problem constants (hardcoded per contract) ----------
N = 360            # color mesh size
P = 120            # partition chunk (N = 3*P)
NCH = 3            # chunks
B = 256            # total batch
NCORES = 8
BL = B // NCORES   # per-core batch
T_MAX = 1000.0
KAPPA = 400.0      # 1/SIGMA_INIT^2
EPS = 1e-5
TWO_PI = 6.283185307179586
# ln(1/(2*pi*i0e(400)))  [i0e(400) = 0.019953356281939987]
LNC = 2.076480848703078
# cos(sqrt(u)) on u in [0, pi^2] (|delta| folded to [0,pi]), power basis c0..c8
COS_COEF = [1.00000000e+00, -5.00000000e-01, 4.16666666e-02, -1.38888885e-03,
            2.48015646e-05, -2.75566515e-07, 2.08651966e-09, -1.13535474e-11,
            4.13131734e-14]

_COMPILED = {}


def _taylor_deg(x, tol, lo):
    """Smallest d with x^(d+1)/(d+1)! < tol."""
    d = lo
    term = x ** (d + 1) / math.factorial(d + 1)
    while term > tol and d < 40:
        d += 1
        term *= x / (d + 1)
    return d


def _plan(anorm):
    """Choose (k_bits, deg_p, deg_r) from ||A||_inf.  The time grid is
    T0 = T_MAX/2^k_bits, chosen so the prelude Taylor converges fast at T0;
    every squaring level applies one bit of the quantized delay."""
    xa = T_MAX * float(anorm)
    if xa <= 0.0:
        return 2, 4, 3
    k0 = max(2, min(16, math.ceil(math.log2(max(xa / 0.9, 2.0)))))

    def degrees(k):
        x0 = xa / (1 << k)
        # prelude truncation amplifies roughly 2^(k/2) through the
        # squarings, so its tolerance adapts to chain depth; the residual
        # Taylor is applied once (no amplification) and tolerates more.
        # Tolerances sized so truncation stays well under the fp32r
        # rounding noise of the chain (~3e-3 at the loss level).
        tol_p = min(max(3e-3 / 2 ** (k / 2), 5e-8), 2e-4)
        return _taylor_deg(x0, tol_p, 3), _taylor_deg(x0, 3e-4, 2)

    # pick k by explicit cost minimization with measured per-stage costs
    # (fp32r: chain level ~2.7us, prelude step ~2.3us, taylor step ~0.3us)
    best = None
    for k in range(max(2, k0 - 2), min(16, k0 + 3) + 1):
        dp, dr = degrees(k)
        cost = (k - 1) * 2.7 + (dp - 1) * 2.3 + dr * 0.3
        if best is None or cost < best[0]:
            best = (cost, k, dp, dr)
    _, k, deg_p, deg_r = best
    return k, deg_p, deg_r


def _build_bass(k_bits, deg_p, deg_r):
    """Construct the Bass program (SPMD; identical on all 8 cores)."""
    import concourse.tile as tile
    from concourse import bacc, mybir

    F32 = mybir.dt.float32
    F32R = mybir.dt.float32r
    AF = mybir.ActivationFunctionType
    OP = mybir.AluOpType

    nc = bacc.Bacc("TRN2", target_bir_lowering=False, debug=False)

    def din(name, shape, dt=F32):
        return nc.dram_tensor(name, shape, dt, kind="ExternalInput").ap()

    d_x = din("x", [N, N], F32R)   # X = T0*A
    d_xi = din("xi", [N, N], F32R)  # I + X (prelude Taylor start)
    # aux = [q0 | masks | rdk] in one DMA: every HWDGE DMA holds the
    # shared DGE ~625ns, so inputs are consolidated
    AUXW = NCH * BL + k_bits * BL + deg_r * BL
    d_aux = din("aux", [P, AUXW], F32R)
    # four top-bit candidate density blocks out; per-sample selection by
    # the top two delay bits + log + mean happen on host
    d_out = nc.dram_tensor("v", [P, 4 * NCH * BL], F32R,
                           kind="ExternalOutput").ap()

    with tile.TileContext(nc) as tc:
        with (
            tc.tile_pool(name="const", bufs=1) as cpool,
            tc.tile_pool(name="mats", bufs=3) as mpool,
            tc.tile_pool(name="qp", bufs=2) as qpool,
            tc.tile_pool(name="vp", bufs=3) as vpool,
            tc.tile_pool(name="tp", bufs=4) as tpool,
            tc.tile_pool(name="psb", bufs=5, space="PSUM") as psb,
            tc.tile_pool(name="pss", bufs=3, space="PSUM") as pss,
        ):
            # ---- constants ------------------------------------------------
            # chunk 0 of x lands as [P,P] pieces on parallel queues so the
            # on-device XT transposes (and hence the first prelude matmul)
            # start early; other chunks whole
            # chunk 0 first (it gates the XT transposes), chunks 1-2 merged
            XN = cpool.tile([P, NCH * N], F32R, tag="x")
            XT = cpool.tile([P, NCH * N], F32R, tag="xt")
            # HWDGE serializes DMAs across queues (~625ns each), so inputs
            # go on one queue in deadline order: x0, x12, aux, xi
            nc.sync.dma_start(XN[:, 0:N], d_x[0:P, :])
            nc.sync.dma_start(
                XN[:, N:3 * N].rearrange("p (c n) -> p c n", c=2),
                d_x[P:3 * P, :].rearrange("(c p) n -> p c n", c=2))
            AUX = cpool.tile([P, AUXW], F32R, tag="aux")
            nc.sync.dma_start(AUX[:], d_aux[:])
            Q = AUX[:, 0:NCH * BL]
            MSK = AUX[:, NCH * BL: (NCH + k_bits) * BL]
            RDK = AUX[:, (NCH + k_bits) * BL: AUXW]
            # identity for PE transposes (fp32r producers: memset can't
            # write fp32r, so build in f32 then copy)
            E120S = cpool.tile([P, P], F32, tag="e120s")
            nc.vector.memset(E120S[:], 1.0)
            nc.gpsimd.affine_select(
                E120S[:], E120S[:], pattern=[[1, P]], compare_op=OP.is_equal,
                fill=0.0, base=0, channel_multiplier=-1,
            )
            E120 = cpool.tile([P, P], F32R, tag="e120")
            nc.vector.tensor_copy(E120[:], E120S[:])
            # X^T built on device: block (c,i) of XT = transpose of X block
            # (i,c); group i reads the contiguous chunk i of XN
            for i in range(NCH):
                pst = psb.tile([P, N], F32, tag="sq")
                for c in range(NCH):
                    nc.tensor.transpose(
                        pst[:, c * P:(c + 1) * P].bitcast(F32R),
                        XN[:, i * N + c * P: i * N + c * P + P],
                        E120[:],
                    )
                if i % 2 == 0:
                    nc.vector.tensor_copy(XT[:, i * N:(i + 1) * N], pst[:])
                else:
                    nc.scalar.copy(XT[:, i * N:(i + 1) * N], pst[:])

            W = N + BL  # merged chunk width: [M_c | Q_c]

            def mm_group(ps, lhsT_tile, rhs_tile, i, rhs_w, rhs_stride=None):
                # lhsT tiles are i-major: block (c, i) at col i*N + c*P, so
                # output chunk i depends on one contiguous lhsT region
                rs = rhs_w if rhs_stride is None else rhs_stride
                for c in range(NCH):
                    nc.tensor.matmul(
                        ps[:],
                        lhsT=lhsT_tile[:, i * N + c * P: i * N + c * P + P],
                        rhs=rhs_tile[:, c * rs: c * rs + rhs_w],
                        start=(c == 0), stop=(c == NCH - 1),
                    )

            def copy_out(dst_ap, ps, idx, small=False):
                # PSUM->SBUF copies: big 1/4 DVE 3/4 ACT; small (transpose
                # blocks) 1/2-1/2 -- ACT's fixed overhead dominates there
                mod = 2 if small else 4
                if idx % mod == 0:
                    nc.vector.tensor_copy(dst_ap, ps[:])
                else:
                    nc.scalar.copy(dst_ap, ps[:])

            # p0 lives in AUX (Q slice); aux/xi DMAs are emitted after the
            # first XT copies so they don't delay them in the queues

            # ---- prelude: ascending Taylor S = I + sum X^k/k!, with the
            # residual Taylor on p0 (V = Q + rdk_j*(X V), j=deg_r..1, which
            # commutes with the bit applies) MERGED into the prelude
            # matmuls as 32 extra moving columns.  T tiles are MQ-shaped
            # ([T_c | V_c]) so each step's psum carries both products and
            # the V drains never wait on a separate matmul set.  I+X comes
            # from the host, saving the identity memset and 3 wide adds.
            S = mpool.tile([P, NCH * W], F32R, tag="M")
            nc.sync.dma_start(
                S[:].rearrange("p (c w) -> p c w", c=NCH)[:, :, 0:N],
                d_xi[:, :].rearrange("(c p) n -> p c n", c=NCH))
            # residual Taylor state: V = Q + rdk_j*(X V), j=deg_r..1,
            # evaluated as standalone narrow matmul sets deferred by one
            # prelude step so the DVE drains never head-of-line block
            tj = {"j": deg_r, "V": Q, "stride": BL, "off": 0}

            def taylor_step(to_dst=False):
                j = tj["j"]
                if j < 1:
                    return
                Vn = None if to_dst else vpool.tile([P, NCH * BL], F32R,
                                                    tag="V")
                rd = RDK[:, (j - 1) * BL: j * BL]
                for i in range(NCH):
                    ps = pss.tile([P, BL], F32, tag="ap")
                    for c in range(NCH):
                        nc.tensor.matmul(
                            ps[:],
                            lhsT=XT[:, i * N + c * P: i * N + c * P + P],
                            rhs=tj["V"][:, c * tj["stride"] + tj["off"]:
                                        c * tj["stride"] + tj["off"] + BL],
                            start=(c == 0), stop=(c == NCH - 1),
                        )
                    # final step lands straight in the MQ tile's Q slots;
                    # both ops on DVE so they never queue behind Pool's
                    # wide S-adds
                    vs = (S[:, i * W + N: (i + 1) * W] if to_dst
                          else Vn[:, i * BL:(i + 1) * BL])
                    nc.vector.tensor_tensor(vs, ps[:], rd, op=OP.mult)
                    nc.vector.tensor_tensor(vs, vs,
                                            Q[:, i * BL:(i + 1) * BL],
                                            op=OP.add)
                if to_dst:
                    tj["j"] = 0
                else:
                    tj.update(j=j - 1, V=Vn[:], stride=BL, off=0)

            T = XN
            for k in range(2, deg_p + 1):
                # taylor matmuls FIRST on PE: their psum is ready before
                # this step's, so the taylor drain on DVE never delays the
                # next gating drain0
                if k > 2:
                    taylor_step(to_dst=(tj["j"] == 1))
                last = k == deg_p
                Tn = mpool.tile([P, NCH * N], F32R, tag="T")
                for i in range(NCH):
                    ps = psb.tile([P, N], F32, tag="sq")
                    mm_group(ps, XT, T, i, N)
                    # drain i=0 on DVE (it gates the next step's first
                    # matmul), the rest on ACT
                    if i == 0:
                        nc.vector.tensor_scalar(Tn[:, i * N:(i + 1) * N],
                                                ps[:], 1.0 / k, None,
                                                op0=OP.mult)
                    else:
                        nc.scalar.mul(Tn[:, i * N:(i + 1) * N], ps[:], 1.0 / k)
                    # S accumulation alternates DVE/Pool (Pool is 2x
                    # slower on wide ops and would otherwise saturate)
                    seng = nc.vector if (k * NCH + i) % 2 == 0 else nc.gpsimd
                    seng.tensor_tensor(S[:, i * W: i * W + N],
                                       S[:, i * W: i * W + N],
                                       Tn[:, i * N:(i + 1) * N],
                                       op=OP.add)
                T = Tn

            while tj["j"] >= 1:
                taylor_step(to_dst=(tj["j"] == 1))

            ST = mpool.tile([P, NCH * N], F32R, tag="MT")

            tr_rot = {"k": 0}

            def transpose_group(MTt, Mt, i):
                # i-major MT: for output chunk i the 3 transposes read the
                # contiguous blocks (i, c) of M and land in ONE [P, N] psum
                # tile drained by a single wide copy; the next squaring's
                # chunk-0 matmuls depend only on the first copy
                pst = psb.tile([P, N], F32, tag="sq")
                for c in range(NCH):
                    nc.tensor.transpose(
                        pst[:, c * P:(c + 1) * P].bitcast(F32R),
                        Mt[:, i * W + c * P: i * W + c * P + P],
                        E120[:],
                    )
                dst = MTt[:, i * N:(i + 1) * N]
                k = tr_rot["k"] = tr_rot["k"] + 1
                # PSUM drains: only DVE/ACT can read PSUM
                if k % 2 == 0:
                    nc.vector.tensor_copy(dst, pst[:])
                else:
                    nc.scalar.copy(dst, pst[:])

            def transpose_mq(MTt, Mt):
                for i in range(NCH):
                    transpose_group(MTt, Mt, i)

            transpose_mq(ST, S)
            M, MT = S, ST

            def square(Mc, MTc, bit=None):
                # Sn = Mc@Mc; if bit is not None also compute Mc@Q (merged
                # columns) and blend it into Sn's Q slot under the bit mask
                # (arithmetic blend: copy_predicated can't produce fp32r).
                Sn = mpool.tile([P, NCH * W], F32R, tag="M")
                STn = mpool.tile([P, NCH * N], F32R, tag="MT")
                wid = N if bit is None else W
                for i in range(NCH):
                    ps = psb.tile([P, wid], F32, tag="sq")
                    mm_group(ps, MTc, Mc, i, wid, rhs_stride=W)
                    copy_out(Sn[:, i * W: i * W + N], ps[:, :N], i)
                    if bit is not None:
                        qold = Mc[:, i * W + N: (i + 1) * W]
                        dq = tpool.tile([P, BL], F32, tag="blend")
                        nc.vector.tensor_tensor(dq[:], ps[:, N:W], qold,
                                                op=OP.subtract)
                        nc.vector.tensor_tensor(
                            dq[:], dq[:], MSK[:, bit * BL:(bit + 1) * BL],
                            op=OP.mult)
                        nc.gpsimd.tensor_tensor(
                            Sn[:, i * W + N: (i + 1) * W], dq[:], qold,
                            op=OP.add)
                transpose_mq(STn, Sn)
                return Sn, STn

            # ---- merged bit applies + chain squarings ---------------------
            # level j squares M (= expm(2^j T0 A)) and applies bit j of the
            # quantized delay to Q in the same matmul set.  The top TWO bits
            # need no further squaring: bit k-2 is a single apply of M_{k-2}
            # and bit k-1 a double apply (M_{k-1} Q = M_{k-2} (M_{k-2} Q)),
            # which is ~2x cheaper than materializing M_{k-1}.
            for j in range(k_bits - 2):
                M, MT = square(M, MT, bit=j)

            QW = NCH * BL

            def apply_wide(q_rhs, rhs_stride, rhs_off):
                # one [P, NCH*BL] psum: chunk i's accumulation lands in
                # slice i, so the blend afterwards is one set of wide ops
                ps = pss.tile([P, QW], F32, tag="ap")
                for i in range(NCH):
                    for c in range(NCH):
                        nc.tensor.matmul(
                            ps[:, i * BL:(i + 1) * BL],
                            lhsT=MT[:, i * N + c * P: i * N + c * P + P],
                            rhs=q_rhs[:, c * rhs_stride + rhs_off:
                                      c * rhs_stride + rhs_off + BL],
                            start=(c == 0), stop=(c == NCH - 1),
                        )
                return ps

            def ccb(ap):
                return ap.rearrange("p (c b) -> p c b", c=NCH)

            # top-2-bit candidates: YOUT = [Q' | MQ' | M^2Q' | M^3Q'] with
            # Q' the fully bit-merged density; the host selects per sample
            # by the top two delay bits, so no device-side blending
            qmq = M[:].rearrange("p (c w) -> p c w", c=NCH)[:, :, N:W]
            YOUT = qpool.tile([P, 4 * QW], F32R, tag="yout")
            nc.gpsimd.tensor_copy(ccb(YOUT[:, 0:QW]), qmq)
            for a in range(1, 4):
                if a == 1:
                    ps = apply_wide(M, W, N)
                else:
                    ps = apply_wide(YOUT, BL, (a - 1) * QW)
                if a == 2:
                    nc.scalar.copy(YOUT[:, a * QW:(a + 1) * QW], ps[:])
                else:
                    nc.vector.tensor_copy(YOUT[:, a * QW:(a + 1) * QW], ps[:])
                if a == 2:
                    # ship the first three candidates while Y3 computes
                    nc.sync.dma_start(d_out[:, 0:3 * QW], YOUT[:, 0:3 * QW])
            nc.sync.dma_start(d_out[:, 3 * QW:4 * QW],
                              YOUT[:, 3 * QW:4 * QW])

    nc.compile()
    return nc


def _host_prep(c_mesh, gtheta, sigma_diff, init_color, delay_t, report_color):
    """Host-side glue: operator assembly (replicating reference f32 ops),
    plan selection, and per-core index/bit/layout arrays."""
    f32 = np.float32
    c = np.asarray(c_mesh, dtype=f32)
    g = np.asarray(gtheta, dtype=f32)
    s = np.asarray(sigma_diff, dtype=f32)[0]
    init = np.asarray(init_color, dtype=f32)
    t = np.asarray(delay_t, dtype=f32)
    rep = np.asarray(report_color, dtype=f32)

    d = (c[1] - c[0]).astype(f32)
    eye = np.eye(N, dtype=f32)
    up = np.roll(eye, -1, axis=1)
    dn = np.roll(eye, 1, axis=1)
    D1 = ((up - dn) / (f32(2.0) * d)).astype(f32)
    D2 = ((up - f32(2.0) * eye + dn) / (d * d)).astype(f32)
    A = ((s ** f32(2.0)) / f32(2.0) * D2 - D1 * g[None, :]).astype(f32)

    anorm = np.abs(A.astype(np.float64)).sum(axis=1).max()
    k_bits, deg_p, deg_r = plan = _plan(anorm)
    T0 = T_MAX / (1 << k_bits)
    X = (A * f32(T0)).astype(f32)

    m = np.floor(t.astype(np.float64) / T0).astype(np.int64)
    m = np.clip(m, 0, (1 << k_bits) - 1)
    r = (t.astype(np.float64) - m * T0) / T0  # in X = T0*A units
    bits = ((m[:, None] >> np.arange(k_bits)[None, :]) & 1)     # [B, K]
    idx = np.argmin(np.abs(c[None, :] - rep[:, None]), axis=1)

    # p0 host-side (O(B*n) glue, like the one-hot/argmin prep): von Mises
    # density replicating the reference's f32 formula
    z = np.cos(c[None, :].astype(np.float64)
               - init[:, None].astype(np.float64)) - 1.0
    p0 = (np.exp(KAPPA * z + LNC)).astype(f32)          # [B, n]

    shared = {
        "x": X,
        "xi": (np.eye(N, dtype=f32) + X).astype(f32),
    }
    in_maps = []
    for core in range(NCORES):
        sl = slice(core * BL, (core + 1) * BL)
        # Q layout [P, NCH*BL]: chunk c at cols [c*BL:(c+1)*BL], Q[p,c*BL+b]
        # = p0[b, c*P+p]
        q0 = np.ascontiguousarray(
            p0[sl].reshape(BL, NCH, P).transpose(2, 1, 0).reshape(P, NCH * BL))
        msk = bits[sl].T.reshape(1, k_bits * BL).astype(f32)
        rdk = np.empty((deg_r, BL), f32)
        for k in range(1, deg_r + 1):
            rdk[k - 1] = (r[sl] / k).astype(f32)
        rdk = rdk.reshape(1, deg_r * BL)
        aux = np.concatenate([
            q0,
            np.broadcast_to(msk, (P, k_bits * BL)),
            np.broadcast_to(rdk, (P, deg_r * BL)),
        ], axis=1).astype(f32)
        in_maps.append(dict(shared, aux=np.ascontiguousarray(aux)))
    # per-sample candidate index = #applies of M_{k-2} from the top 2 bits
    topsel = bits[:, k_bits - 2] + 2 * bits[:, k_bits - 1]
    return plan, in_maps, (idx, topsel)


def _get_nc(plan):
    if plan not in _COMPILED:
        _COMPILED[plan] = _build_bass(*plan)
    return _COMPILED[plan]


def kernel(**inputs):
    from concourse.bass_utils import run_bass_kernel_spmd

    plan, in_maps, (idx, topsel) = _host_prep(
        inputs["c_mesh"], inputs["gtheta"], inputs["sigma_diff"],
        inputs["init_color"], inputs["delay_t"], inputs["report_color"],
    )
    nc = _get_nc(plan)
    res = run_bass_kernel_spmd(nc, in_maps, list(range(NCORES)))
    # host-side candidate pick + selection + log + mean (O(B) glue):
    # v[p, a*NCH*BL + c*BL + b] holds candidate a of p1[b, c*P+p]
    QW = NCH * BL
    terms = np.empty(B, np.float64)
    for core in range(NCORES):
        v = np.asarray(res.results[core]["v"])
        for b in range(BL):
            g = core * BL + b
            ix = idx[g]
            p_sel = np.float32(
                v[ix % P, topsel[g] * QW + (ix // P) * BL + b])
            terms[g] = np.log(np.float32(max(p_sel, 0.0) + EPS))
    loss = -np.mean(terms)
    return np.asarray(loss, dtype=np.float32)

